# revision 19
# baseline (speedup 1.0000x reference)
"""MoE layer (N=8192, D=1024, F=4096, E=8, top-2) on 8 Trainium2 NeuronCores.

Strategy (expert-parallel, matches the sharding hint):
  - Host: gate (inputs @ Wg + bg), top-k selection, softmax combine weights,
    and the dispatch/combine index plumbing (gather tokens per expert,
    scatter-add expert outputs back). This is the tiny O(N*D*E) part.
  - Device (SPMD, core e == expert e): the heavy FFN
        y = silu(x_e @ W1[e] + b1[e]) @ W2[e]  scaled per-row by the
    combine weight.

Per-core kernel layout:
  mm1: h^T[f, t] = W1[d, f]^T @ x^T[d, t]   (stationary = W1 tile, moving = x^T)
  silu+bias on ScalarE (PSUM -> SBUF), h^T kept resident in SBUF
  mm2: y[t, d]  = h^T[f, t]^T @ W2[f, d]    (stationary = h^T tile, moving = W2)
  scale rows by combine weight on VectorE, DMA out.
Tokens are processed in blocks of <=768 so mm2 can hold block/128 PSUM
accumulators per 512-wide half of D.

Two device variants:
  - "bf16_resident" (default): weights converted to bf16 on host and kept
    fully resident in SBUF (64+64 KB/partition); activations bf16; fp32
    PSUM accumulate.  Fast LDWEIGHTS (FWL), no weight re-streaming.
  - "f32r_stream": everything fp32r (full-rate fp32 matmul); weights are
    re-streamed per token block.  ~10x more accurate, somewhat slower.
"""

import os
import sys
import types

import numpy as np

import concourse.bass as bass
import concourse.bacc as bacc
import concourse.mybir as mybir
import concourse.tile as tile
from concourse.bass_utils import run_bass_kernel_spmd


def _ensure_ntff_hook():
    """Provide antenv.axon_hooks if the image lacks it, so trace=True (or a
    caller-set BASS_TRACE=1) degrades gracefully instead of crashing in
    run_bass_kernel_spmd. Uses the same ctypes NTFF hook the axon boot
    would install when available."""
    try:
        import antenv.axon_hooks  # noqa: F401

        return
    except ImportError:
        pass
    hook = None
    try:
        from trn_agent_boot.trn_boot import _ntff_profile_via_ctypes

        hook = _ntff_profile_via_ctypes("/opt/axon/libaxon_pjrt.so")
    except Exception:
        hook = None
    m = types.ModuleType("antenv.axon_hooks")
    m.get_axon_ntff_profile_hook = lambda: hook
    m.set_axon_ntff_profile_hook = lambda h: None
    sys.modules["antenv.axon_hooks"] = m
    try:
        import antenv

        antenv.axon_hooks = m
    except ImportError:
        pass


_ensure_ntff_hook()

F32 = mybir.dt.float32
F32R = mybir.dt.float32r
BF16 = mybir.dt.bfloat16

D_MODEL = 1024
D_FF = 4096
N_EXPERTS = 8
N_CORES = 8
MAX_BLK = 768  # tokens per block; block/128 PSUM banks used in mm2 per D-half

MODE = os.environ.get("MOE_KERNEL_MODE", "bf16_resident")

# exec time (ns) of the most recent device run, when tracing was enabled
LAST_EXEC_TIME_NS = None
_NC_CACHE = {}


def _split_blocks(C):
    """Split C (multiple of 128) into blocks of at most MAX_BLK tokens."""
    blocks = []
    t = C
    while t > 0:
        b = min(t, MAX_BLK)
        blocks.append(b)
        t -= b
    return blocks


def _split_subtiles(blk):
    """Split a block into moving-dim subtiles <=512 (one PSUM bank)."""
    out = []
    t = blk
    while t > 0:
        s = min(t, 512)
        out.append(s)
        t -= s
    return out


def _build_nc_bf16_resident(C):
    """bf16 weights fully resident in SBUF; bf16 activations; f32 psum.

    Host pre-shuffles all inputs to partition-major chunk layouts so every
    DMA is 128 fully-contiguous descriptors:
      w1: [8, 128, 8, 512]   (f-chunk, partition, d-chunk, f-within)
      w2: [4, 128, 8, 1024]  (f-chunk, partition, f-within, d)
      x:  [nb, 128, 8, 768]  (block, partition, d-chunk, token)
      b1: [128, 32]  cw: [128, C/128]
    """
    nc = bacc.Bacc("TRN2", target_bir_lowering=False, debug=False)
    D, F = D_MODEL, D_FF
    nf = F // 128  # 32
    nd = D // 128  # 8
    blocks = _split_blocks(C)
    nb = len(blocks)

    w1 = nc.declare_dram_parameter("w1", [16, 128, nd, F // 16], BF16, isOutput=False)
    w2 = nc.declare_dram_parameter("w2", [4, 128, nf // 4, D], BF16, isOutput=False)
    xT = nc.declare_dram_parameter("xT", [nb, 128, nd, MAX_BLK], BF16, isOutput=False)
    b1 = nc.declare_dram_parameter("b1", [128, nf], F32, isOutput=False)
    cw = nc.declare_dram_parameter("cw", [128, C // 128], F32, isOutput=False)
    y = nc.declare_dram_parameter("y", [C, D], F32, isOutput=True)

    with tile.TileContext(nc) as tc:
        with (
            tc.tile_pool(name="const", bufs=1) as constp,
            tc.tile_pool(name="wres", bufs=1) as wres,
            tc.tile_pool(name="xp", bufs=1) as xp,
            tc.tile_pool(name="hp", bufs=1) as hp,
            tc.tile_pool(name="yp", bufs=3) as yp,
            tc.tile_pool(name="ps1", bufs=2, space="PSUM") as ps1,
            tc.tile_pool(name="ps2", bufs=6, space="PSUM") as ps2,
        ):
            w1_sb = wres.tile([128, 16, nd, F // 16], BF16, tag="w1")
            w2_sb = wres.tile([128, 4, nf // 4, D], BF16, tag="w2")
            x_first = xp.tile([128, nd, MAX_BLK], BF16, tag="x")
            # first w1 chunk + first x half-block gate the first matmuls
            # w1 in 16 small sub-chunks on sync (HWDGE): each DIRECT2D is
            # short, so the sequencer can interleave DMA-completion relays
            # instead of blocking on queue backpressure for 20+ us
            # keep sync's descriptor-issue queue short: only the DMAs the
            # first octets need; sync must stay free to relay completions
            nc.sync.dma_start(w1_sb[:, 0], w1[0])
            xs0 = min(512, blocks[0])
            nc.sync.dma_start(x_first[:, :, :xs0], xT[0][:, :, :xs0])
            nc.sync.dma_start(w1_sb[:, 1], w1[1])
            if blocks[0] > 512:
                nc.sync.dma_start(x_first[:, :, 512:], xT[0][:, :, 512:])
            b1_sb = constp.tile([128, nf], F32, tag="b1")
            nc.gpsimd.dma_start(b1_sb[:], b1[:])
            cw_sb = constp.tile([128, C // 128], F32, tag="cw")
            nc.gpsimd.dma_start(cw_sb[:], cw[:])
            # near-term w1 chunks from scalar (the other HWDGE engine, idle
            # until the first silu); far chunks + w2 from gpsimd (SWDGE is
            # slow but those aren't needed for 40+ us)
            for c in range(2, 8):
                nc.scalar.dma_start(w1_sb[:, c], w1[c])
            for c in range(8, 16):
                nc.gpsimd.dma_start(w1_sb[:, c], w1[c])
            for c in range(4):
                nc.gpsimd.dma_start(w2_sb[:, c], w2[c])

            t0 = 0
            for bi, blk in enumerate(blocks):
                ntt = blk // 128
                if bi == 0:
                    x_sb = x_first
                else:
                    x_sb = xp.tile([128, nd, MAX_BLK], BF16, tag="x")
                    nc.sync.dma_start(x_sb[:], xT[bi])
                h_sb = hp.tile([128, nf, MAX_BLK], BF16, tag="h")

                # ---- phase 1: h^T = silu(W1^T x^T + b1) ----
                for f in range(nf):
                    s0 = 0
                    for ts in _split_subtiles(blk):
                        ph = ps1.tile([128, 512], F32, tag="ph")
                        for d in range(nd):
                            nc.tensor.matmul(
                                ph[:, :ts],
                                w1_sb[:, f // 2, d, (f % 2) * 128 : (f % 2 + 1) * 128],
                                x_sb[:, d, s0 : s0 + ts],
                                start=(d == 0),
                                stop=(d == nd - 1),
                            )
                        nc.scalar.activation(
                            h_sb[:, f, s0 : s0 + ts],
                            ph[:, :ts],
                            mybir.ActivationFunctionType.Silu,
                            bias=b1_sb[:, f : f + 1],
                        )
                        s0 += ts

                # ---- phase 2: y = (h^T)^T W2, scaled by combine weight ----
                for dh in range(2):
                    pys = [
                        ps2.tile([128, 512], F32, tag="py", name=f"py{i}")
                        for i in range(ntt)
                    ]
                    for f in range(nf):
                        for tt in range(ntt):
                            nc.tensor.matmul(
                                pys[tt][:],
                                h_sb[:, f, tt * 128 : (tt + 1) * 128],
                                w2_sb[:, f // 8, f % 8, dh * 512 : (dh + 1) * 512],
                                start=(f == 0),
                                stop=(f == nf - 1),
                            )
                    for tt in range(ntt):
                        g = t0 // 128 + tt
                        y_sb = yp.tile([128, 512], F32, tag="y")
                        nc.vector.tensor_scalar_mul(
                            y_sb[:], pys[tt][:], cw_sb[:, g : g + 1]
                        )
                        nc.sync.dma_start(
                            y[t0 + tt * 128 : t0 + (tt + 1) * 128,
                              dh * 512 : (dh + 1) * 512],
                            y_sb[:],
                        )
                t0 += blk
    nc.finalize()  # Bacc: runs wait-legalization + register allocation
    return nc


def _build_nc_f32r_stream(C):
    """All-fp32r variant; weights re-streamed per token block.

    Host layouts (partition-major, fully contiguous DMAs):
      w1: [32, 128, 8, 128]  (f-tile, partition, d-chunk, f-within)
      w2: [32, 2, 128, 512]  (f-tile, d-half, partition, d-within)
      x:  [nb, 128, 8, 768]  b1: [128, 32]  cw: [128, C/128]
    """
    nc = bacc.Bacc("TRN2", target_bir_lowering=False, debug=False)
    D, F = D_MODEL, D_FF
    nf = F // 128
    nd = D // 128
    blocks = _split_blocks(C)
    nb = len(blocks)

    w1 = nc.declare_dram_parameter("w1", [nf, 128, nd, 128], F32R, isOutput=False)
    w2 = nc.declare_dram_parameter("w2", [nf, 2, 128, 512], F32R, isOutput=False)
    xT = nc.declare_dram_parameter("xT", [nb, 128, nd, MAX_BLK], F32R, isOutput=False)
    b1 = nc.declare_dram_parameter("b1", [128, nf], F32, isOutput=False)
    cw = nc.declare_dram_parameter("cw", [128, C // 128], F32, isOutput=False)
    y = nc.declare_dram_parameter("y", [C, D], F32, isOutput=True)

    with tile.TileContext(nc) as tc:
        with (
            tc.tile_pool(name="const", bufs=1) as constp,
            tc.tile_pool(name="xp", bufs=2) as xp,
            tc.tile_pool(name="hp", bufs=1) as hp,
            tc.tile_pool(name="w1p", bufs=4) as w1p,
            tc.tile_pool(name="w2p", bufs=8) as w2p,
            tc.tile_pool(name="yp", bufs=3) as yp,
            tc.tile_pool(name="ps1", bufs=2, space="PSUM") as ps1,
            tc.tile_pool(name="ps2", bufs=6, space="PSUM") as ps2,
        ):
            b1_sb = constp.tile([128, nf], F32, tag="b1")
            nc.gpsimd.dma_start(b1_sb[:], b1[:])
            cw_sb = constp.tile([128, C // 128], F32, tag="cw")
            nc.gpsimd.dma_start(cw_sb[:], cw[:])

            t0 = 0
            for bi, blk in enumerate(blocks):
                ntt = blk // 128
                x_sb = xp.tile([128, nd, MAX_BLK], F32R, tag="x")
                xs0 = min(512, blk)
                nc.sync.dma_start(x_sb[:, :, :xs0], xT[bi][:, :, :xs0])
                if blk > 512:
                    nc.sync.dma_start(x_sb[:, :, 512:blk], xT[bi][:, :, 512:blk])
                h_sb = hp.tile([128, nf, MAX_BLK], F32R, tag="h")

                # ---- phase 1 ----
                for f in range(nf):
                    w1_sb = w1p.tile([128, nd, 128], F32R, tag="w1")
                    nc.sync.dma_start(w1_sb[:], w1[f])
                    s0 = 0
                    for ts in _split_subtiles(blk):
                        ph = ps1.tile([128, 512], F32, tag="ph")
                        for d in range(nd):
                            nc.tensor.matmul(
                                ph[:, :ts],
                                w1_sb[:, d, :],
                                x_sb[:, d, s0 : s0 + ts],
                                start=(d == 0),
                                stop=(d == nd - 1),
                            )
                        nc.scalar.activation(
                            h_sb[:, f, s0 : s0 + ts],
                            ph[:, :ts],
                            mybir.ActivationFunctionType.Silu,
                            bias=b1_sb[:, f : f + 1],
                        )
                        s0 += ts

                # ---- phase 2 ----
                for dh in range(2):
                    pys = [
                        ps2.tile([128, 512], F32, tag="py", name=f"py{i}")
                        for i in range(ntt)
                    ]
                    for f in range(nf):
                        w2_sb = w2p.tile([128, 512], F32R, tag="w2")
                        nc.gpsimd.dma_start(w2_sb[:], w2[f, dh])
                        for tt in range(ntt):
                            nc.tensor.matmul(
                                pys[tt][:],
                                h_sb[:, f, tt * 128 : (tt + 1) * 128],
                                w2_sb[:],
                                start=(f == 0),
                                stop=(f == nf - 1),
                            )
                    for tt in range(ntt):
                        g = t0 // 128 + tt
                        y_sb = yp.tile([128, 512], F32, tag="y")
                        nc.vector.tensor_scalar_mul(
                            y_sb[:], pys[tt][:], cw_sb[:, g : g + 1]
                        )
                        nc.sync.dma_start(
                            y[t0 + tt * 128 : t0 + (tt + 1) * 128,
                              dh * 512 : (dh + 1) * 512],
                            y_sb[:],
                        )
                t0 += blk
    nc.finalize()
    return nc


def _route(inputs, Wg, bg, k):
    """Host gate: replicate reference numerics (fp32) for routing."""
    logits = inputs.astype(np.float32) @ Wg.astype(np.float32) + bg.astype(np.float32)
    sel = np.argsort(-logits, axis=1, kind="stable")[:, :k]  # == jax.lax.top_k order
    tl = np.take_along_axis(logits, sel, axis=1).astype(np.float32)
    m = tl.max(axis=1, keepdims=True)
    e = np.exp(tl - m, dtype=np.float32)
    w = (e / e.sum(axis=1, keepdims=True)).astype(np.float32)
    return sel, w


def kernel(inputs, Wg, bg, W1, b1, W2, b2, k):
    global LAST_EXEC_TIME_NS
    k = int(np.asarray(k))
    inputs = np.ascontiguousarray(np.asarray(inputs, dtype=np.float32))
    Wg = np.asarray(Wg, dtype=np.float32)
    bg = np.asarray(bg, dtype=np.float32)
    W1 = np.asarray(W1, dtype=np.float32)
    b1 = np.asarray(b1, dtype=np.float32)
    W2 = np.asarray(W2, dtype=np.float32)
    b2 = np.asarray(b2, dtype=np.float32)

    N, D = inputs.shape
    E = Wg.shape[1]
    assert E == N_EXPERTS and D == D_MODEL and W1.shape == (E, D, D_FF)

    sel, w = _route(inputs, Wg, bg, k)

    # per-expert token lists
    idxs, wvals = [], []
    for e in range(E):
        tok, slot = np.nonzero(sel == e)
        idxs.append(tok)
        wvals.append(w[tok, slot])
    max_cnt = max(len(ix) for ix in idxs)
    C = max(((max_cnt + 127) // 128) * 128, 128)

    if MODE == "bf16_resident":
        import ml_dtypes

        wdt = ml_dtypes.bfloat16
    else:
        wdt = np.float32

    in_maps = []
    nb = len(_split_blocks(C))
    Cp = nb * MAX_BLK  # x padded to whole blocks
    for e in range(E):
        cnt = len(idxs[e])
        cwe = np.zeros((C,), dtype=np.float32)
        cwe[:cnt] = wvals[e]
        if MODE == "bf16_resident":
            xe = np.zeros((Cp, D), dtype=wdt)
            xe[:cnt] = inputs[idxs[e]].astype(wdt)
            # [Cp, D] -> [nb, 128, 8, MAX_BLK]: t=(b, t'), d=(a, p)
            xe = np.ascontiguousarray(
                xe.reshape(nb, MAX_BLK, 8, 128).transpose(0, 3, 2, 1)
            )
            w1e = np.ascontiguousarray(
                W1[e].astype(wdt).reshape(8, 128, 16, 256).transpose(2, 1, 0, 3)
            )  # [fc, p, d-chunk, f-within]
            w2e = np.ascontiguousarray(
                W2[e].astype(wdt).reshape(4, 8, 128, D).transpose(0, 2, 1, 3)
            )  # [fc, p, f-within, d]
            b1e = np.ascontiguousarray(b1[e].reshape(32, 128).T)
            cwe = np.ascontiguousarray(cwe.reshape(C // 128, 128).T)
        else:
            xe = np.zeros((Cp, D), dtype=wdt)
            xe[:cnt] = inputs[idxs[e]]
            xe = np.ascontiguousarray(
                xe.reshape(nb, MAX_BLK, 8, 128).transpose(0, 3, 2, 1)
            )
            w1e = np.ascontiguousarray(
                W1[e].reshape(8, 128, 32, 128).transpose(2, 1, 0, 3)
            )  # [f-tile, p, d-chunk, f-within]
            w2e = np.ascontiguousarray(
                W2[e].reshape(32, 128, 2, 512).transpose(0, 2, 1, 3)
            )  # [f-tile, d-half, p, d-within]
            b1e = np.ascontiguousarray(b1[e].reshape(32, 128).T)
            cwe = np.ascontiguousarray(cwe.reshape(C // 128, 128).T)
        in_maps.append(
            {"xT": xe, "w1": w1e, "b1": b1e, "w2": w2e, "cw": cwe}
        )

    key = (MODE, C)
    if key not in _NC_CACHE:
        if MODE == "bf16_resident":
            _NC_CACHE[key] = _build_nc_bf16_resident(C)
        else:
            _NC_CACHE[key] = _build_nc_f32r_stream(C)
    nc = _NC_CACHE[key]

    trace = bool(os.environ.get("BASS_TRACE"))
    res = run_bass_kernel_spmd(nc, in_maps, core_ids=list(range(N_CORES)), trace=trace)
    LAST_EXEC_TIME_NS = getattr(res, "exec_time_ns", None)

    results = np.zeros((N, D), dtype=np.float32)
    for e in range(E):
        cnt = len(idxs[e])
        ye = np.asarray(res.results[e]["y"])[:cnt]
        # device computed w * (silu(x W1 + b1) @ W2); add the w * b2[e] term here
        results[idxs[e]] += ye + wvals[e][:, None] * b2[e][None, :]
    return results.astype(np.float32)
